# revision 15
# baseline (speedup 1.0000x reference)
"""Trainium2 Bass kernel for an 8-layer weight-shared dense transformer variant.

Sharding: data-parallel over batch B=2 x tensor-parallel over the NH=4 heads
(core = 4*b + h). Each core runs the full layer stack for its (b, h) pair;
the yMLP head-contributions are summed with an AllReduce over the 4 cores of
each batch group per layer.

The neuron axis (N=8192) is permuted host-side so that RoPE pairs (2i, 2i+1)
land in separate adjacent 128-row tiles ("even" tile / "odd" tile per group of
256 neurons).  All contractions over n are permutation-invariant, so only the
encoder/encoder_v columns, decoder rows and cos/sin tables need the matching
permutation.  This makes the RoPE rotation pure full-width elementwise ops with
no cross-partition shuffles.
"""

import math
import os

import ml_dtypes
import numpy as np

import concourse.bass as bass
import concourse.mybir as mybir
import concourse.tile as tile
from concourse.bass_utils import run_bass_kernel_spmd
from concourse.masks import make_causal_mask, make_identity

BF16 = mybir.dt.bfloat16
F32 = mybir.dt.float32
AF = mybir.ActivationFunctionType
ALU = mybir.AluOpType

N_CORES = 8
B, T, D, NH, N, VOCAB = 2, 1024, 256, 4, 8192, 256
NPAIR = N // 2
NTILES = N // 128  # 64
NGROUP = NPAIR // 128  # 32 pair groups
TBLK = T // 128  # 8
EPS = 1e-5
THETA = 2.0 ** 16
N_LAYER = int(os.environ.get("KERNEL_N_LAYER", "8"))
ABLATE_DMA = os.environ.get("KERNEL_ABLATE_DMA", "0") == "1"
RECOMPUTE_XSP = os.environ.get("KERNEL_RECOMPUTE_XSP", "0") == "1"
ABLATE_CC = os.environ.get("KERNEL_ABLATE_CC", "0") == "1"

REPLICA_GROUPS = [[0, 1, 2, 3], [4, 5, 6, 7]]


# ---------------------------------------------------------------- host side

def _split_multiwaits_json(bir: bytes) -> bytes:
    """This walrus build rejects instructions carrying more than one sync-wait
    ("Too many sync wait commands"), while Tile freely attaches several.
    Split: hoist all but the last wait of each instruction onto NoOps inserted
    immediately before it (same engine => executes right before it, so the
    AND-of-waits semantics is preserved)."""
    import json

    m = json.loads(bir)
    n_fixed = 0
    for func in m["functions"]:
        for blk in func["blocks"]:
            insts = blk["instructions"]
            out = []
            for inst in insts:
                si = inst.get("sync_info")
                waits = (si or {}).get("on_wait") or []
                if len(waits) > 1:
                    for k, w in enumerate(waits[:-1]):
                        out.append({
                            "engine": inst["engine"],
                            "ins": [],
                            "name": f"hw_{inst['name']}_{k}",
                            "opcode": "NoOp",
                            "outs": [],
                            "sync_info": {"on_update": [], "on_wait": [w]},
                        })
                    si["on_wait"] = [waits[-1]]
                    n_fixed += 1
                out.append(inst)
            blk["instructions"] = out
    return json.dumps(m).encode()


def _install_json_fix(nc):
    orig = nc.to_json_bytes

    def patched():
        return _split_multiwaits_json(orig())

    nc.to_json_bytes = patched


def _pair_perm():
    """new index k -> original n.  Group j (256 original neurons) becomes an
    'even' tile (rows 2r) followed by an 'odd' tile (rows 2r+1)."""
    perm = np.empty(N, dtype=np.int64)
    r = np.arange(128)
    for j in range(NGROUP):
        base = 256 * j
        perm[base: base + 128] = base + 2 * r
        perm[base + 128: base + 256] = base + 2 * r + 1
    return perm


def _tables():
    """cos/sin of 2*pi*((t*f) mod 1) for pair row p (=128j+r), f64 -> bf16."""
    p = np.arange(NPAIR, dtype=np.float64)
    f = 1.0 / (THETA ** (2.0 * p / N)) / (2.0 * math.pi)
    t = np.arange(T, dtype=np.float64)
    ang = 2.0 * math.pi * np.mod(t[None, :] * f[:, None], 1.0)
    return (
        np.cos(ang).astype(ml_dtypes.bfloat16),
        np.sin(ang).astype(ml_dtypes.bfloat16),
    )


def _ln_np(x):
    x = x.astype(np.float64)
    mu = x.mean(-1, keepdims=True)
    var = ((x - mu) ** 2).mean(-1, keepdims=True)
    return ((x - mu) / np.sqrt(var + EPS)).astype(np.float32)


# ---------------------------------------------------------------- bass build

def build_nc(n_layers=N_LAYER, debug_dump=None):
    debug_dump = (os.environ.get("KERNEL_DEBUG", "0") == "1"
                  if debug_dump is None else debug_dump)
    nc = bass.Bass("TRN2", target_bir_lowering=False, debug=False,
                   num_devices=N_CORES)
    dbg = {}
    if debug_dump:
        dbg["xt0"] = nc.dram_tensor("dbg_xt0", [128, 2, T], BF16,
                                    kind="ExternalOutput").ap()
        dbg["xsp01"] = nc.dram_tensor("dbg_xsp01", [2, 128, T], BF16,
                                      kind="ExternalOutput").ap()
        dbg["qrt01"] = nc.dram_tensor("dbg_qrt01", [2, 128, T], BF16,
                                      kind="ExternalOutput").ap()
        dbg["sc"] = nc.dram_tensor("dbg_sc", [128, 512], BF16,
                                   kind="ExternalOutput").ap()
        dbg["ykv"] = nc.dram_tensor("dbg_ykv", [128, TBLK, D], BF16,
                                    kind="ExternalOutput").ap()
        dbg["ykvt"] = nc.dram_tensor("dbg_ykvt", [128, 2, T], BF16,
                                     kind="ExternalOutput").ap()
        dbg["m0"] = nc.dram_tensor("dbg_m0", [128, T], BF16,
                                   kind="ExternalOutput").ap()
        dbg["ymlp_pre"] = nc.dram_tensor("dbg_ymlp_pre", [128, TBLK, D], F32,
                                         kind="ExternalOutput").ap()
        dbg["ymlp_post"] = nc.dram_tensor("dbg_ymlp_post", [128, TBLK, D], F32,
                                          kind="ExternalOutput").ap()
        dbg["x1"] = nc.dram_tensor("dbg_x1", [128, TBLK, D], F32,
                                   kind="ExternalOutput").ap()

    x0_d = nc.dram_tensor("x0", [T, D], F32, kind="ExternalInput")
    enc_d = nc.dram_tensor("enc", [D, N], BF16, kind="ExternalInput")
    encv_d = nc.dram_tensor("encv", [D, N], BF16, kind="ExternalInput")
    dec_d = nc.dram_tensor("dec", [N, D], BF16, kind="ExternalInput")
    cos_d = nc.dram_tensor("costab", [NPAIR, T], BF16, kind="ExternalInput")
    sin_d = nc.dram_tensor("sintab", [NPAIR, T], BF16, kind="ExternalInput")
    lmh_d = nc.dram_tensor("lmh", [D, VOCAB], BF16, kind="ExternalInput")
    out_d = nc.dram_tensor("logits", [T, VOCAB], F32, kind="ExternalOutput")
    xsp_d = nc.dram_tensor("xsp_scratch", [NTILES, 128, T], BF16).ap()

    enc_r = enc_d.ap().rearrange("(dh p) n -> p dh n", p=128)
    encv_r = encv_d.ap().rearrange("(dh p) n -> p dh n", p=128)
    dec_r = dec_d.ap().rearrange("(nt p) d -> p nt d", p=128)
    x0_r = x0_d.ap().rearrange("(i p) d -> p i d", p=128)
    lmh_r = lmh_d.ap().rearrange("(dh p) v -> p dh v", p=128)
    out_r = out_d.ap().rearrange("(i p) v -> p i v", p=128)
    cos_a, sin_a = cos_d.ap(), sin_d.ap()

    from contextlib import ExitStack

    with tile.TileContext(nc) as tc, ExitStack() as ctx:
        const = ctx.enter_context(tc.tile_pool(name="const", bufs=1))
        xpool = ctx.enter_context(tc.tile_pool(name="xpool", bufs=2))
        xbfp = ctx.enter_context(tc.tile_pool(name="xbfp", bufs=1))
        xtp = ctx.enter_context(tc.tile_pool(name="xtp", bufs=1))
        stagep = ctx.enter_context(tc.tile_pool(name="stagep", bufs=1))
        lnp = ctx.enter_context(tc.tile_pool(name="lnp", bufs=2))
        stats = ctx.enter_context(tc.tile_pool(name="stats", bufs=4))
        dram = ctx.enter_context(tc.tile_pool(name="dram", bufs=2, space="DRAM"))

        ident = const.tile([128, 128], BF16, name="ident")
        make_identity(nc, ident)
        ident32 = const.tile([128, 128], F32, name="ident32")
        make_identity(nc, ident32)
        trim = const.tile([128, 128], F32, name="trim")
        make_causal_mask(nc, trim, mask_val=1.0)  # 1.0 where t > s else 0.0
        lmh_t = const.tile([128, 2, VOCAB], BF16, name="lmh_t")
        nc.sync.dma_start(out=lmh_t, in_=lmh_r)
        eps_t = const.tile([128, 1], F32, name="eps_t")
        nc.vector.memset(eps_t, EPS)

        def transpose_128(dst, src, psum_pool):
            """dst[:, j, i] = src[:, i, j] for one (128,128) bf16 block."""
            pst = psum_pool.tile([128, 128], BF16, name="tp", tag="tp")
            nc.tensor.transpose(out=pst, in_=src, identity=ident)
            nc.scalar.copy(out=dst, in_=pst)

        def make_x_forms(x_f32, psum_pool):
            x_bf = xbfp.tile([128, TBLK, D], BF16, name="x_bf", tag="x_bf")
            nc.scalar.copy(out=x_bf, in_=x_f32)
            xT = xtp.tile([128, 2, T], BF16, name="xT", tag="xT")
            for i in range(TBLK):
                for dh in range(2):
                    transpose_128(
                        xT[:, dh, i * 128:(i + 1) * 128],
                        x_bf[:, i, dh * 128:(dh + 1) * 128],
                        psum_pool,
                    )
            return x_bf, xT

        def layer_norm_tile(dst, src, out_dtype_hint=None):
            """LN over the last (free, 256) dim of one (128, 256) tile."""
            st = stats.tile([128, 6], F32, name="bn_st", tag="bn_st")
            mv = stats.tile([128, 2], F32, name="bn_mv", tag="bn_mv")
            rs = stats.tile([128, 1], F32, name="rstd", tag="rstd")
            nc.vector.bn_stats(out=st, in_=src)
            nc.vector.bn_aggr(out=mv, in_=st)
            nc.scalar.activation(out=rs, in_=mv[:, 1:2], func=AF.Sqrt,
                                 bias=eps_t, scale=1.0)
            nc.vector.reciprocal(rs, rs)
            nc.vector.tensor_scalar(
                out=dst, in0=src, scalar1=mv[:, 0:1], scalar2=rs,
                op0=ALU.subtract, op1=ALU.mult,
            )

        # initial x forms
        x_f32 = xpool.tile([128, TBLK, D], F32, name="x", tag="x")
        nc.sync.dma_start(out=x_f32, in_=x0_r)
        with tc.tile_pool(name="ipsum", bufs=2, space="PSUM") as ipsum:
            x_bf, xT = make_x_forms(x_f32, ipsum)
        if dbg:
            nc.sync.dma_start(out=dbg["xt0"], in_=xT)

        for li in range(n_layers):
            # ---------------- phase A: x_sparse + rope -> QRT (sbuf resident)
            with ExitStack() as lctx:
                qrt_p = lctx.enter_context(tc.tile_pool(name="qrt", bufs=1))
                QRT = qrt_p.tile([128, NTILES, T], BF16, name="QRT", tag="QRT")
                with ExitStack() as actx:
                    encs = actx.enter_context(tc.tile_pool(name="encs", bufs=3))
                    tabs = actx.enter_context(tc.tile_pool(name="tabs", bufs=4))
                    xsps = actx.enter_context(tc.tile_pool(name="xsps", bufs=4))
                    ropet = actx.enter_context(tc.tile_pool(name="ropet", bufs=2))
                    apsum = actx.enter_context(
                        tc.tile_pool(name="apsum", bufs=4, space="PSUM"))
                    for j in range(NGROUP):
                        enc_c = encs.tile([128, 2, 256], BF16, name="enc_c",
                                          tag="enc_c")
                        nc.sync.dma_start(
                            out=enc_c, in_=enc_r[:, :, j * 256:(j + 1) * 256])
                        if not ABLATE_DMA or j == 0:
                            cos_t = tabs.tile([128, T], BF16, name="cos_t",
                                              tag="cos_t")
                            sin_t = tabs.tile([128, T], BF16, name="sin_t",
                                              tag="sin_t")
                            nc.sync.dma_start(out=cos_t,
                                              in_=cos_a[j * 128:(j + 1) * 128, :])
                            nc.sync.dma_start(out=sin_t,
                                              in_=sin_a[j * 128:(j + 1) * 128, :])
                            abl_cos, abl_sin = cos_t, sin_t
                        else:
                            cos_t, sin_t = abl_cos, abl_sin
                        xsp_sb = []
                        for par in range(2):
                            nt = 2 * j + par
                            xsp_t = xsps.tile([128, T], BF16, name="xsp_t",
                                              tag=f"xsp_{par}")
                            for th in range(2):
                                ps = apsum.tile([128, 512], F32, name="aps",
                                                tag="aps")
                                nc.tensor.matmul(
                                    ps,
                                    lhsT=enc_c[:, 0, par * 128:(par + 1) * 128],
                                    rhs=xT[:, 0, th * 512:(th + 1) * 512],
                                    start=True, stop=False)
                                nc.tensor.matmul(
                                    ps,
                                    lhsT=enc_c[:, 1, par * 128:(par + 1) * 128],
                                    rhs=xT[:, 1, th * 512:(th + 1) * 512],
                                    start=False, stop=True)
                                nc.scalar.activation(
                                    out=xsp_t[:, th * 512:(th + 1) * 512],
                                    in_=ps, func=AF.Relu)
                            if not ABLATE_DMA and not RECOMPUTE_XSP:
                                nc.sync.dma_start(out=xsp_d[nt], in_=xsp_t)
                            if dbg and li == 0 and j == 0:
                                nc.sync.dma_start(out=dbg["xsp01"][par],
                                                  in_=xsp_t)
                            xsp_sb.append(xsp_t)
                        for dst, s0, s1, sub in ((2 * j, 0, 1, True),
                                                 (2 * j + 1, 1, 0, False)):
                            a = ropet.tile([128, T], BF16, name="ra", tag="ra")
                            bb = ropet.tile([128, T], BF16, name="rb", tag="rb")
                            nc.vector.tensor_mul(a, xsp_sb[s0], cos_t)
                            nc.vector.tensor_mul(bb, xsp_sb[s1], sin_t)
                            if sub:
                                nc.vector.tensor_sub(QRT[:, dst, :], a, bb)
                            else:
                                nc.vector.tensor_add(QRT[:, dst, :], a, bb)
                        if dbg and li == 0 and j == 0:
                            nc.sync.dma_start(out=dbg["qrt01"][0],
                                              in_=QRT[:, 0, :])
                            nc.sync.dma_start(out=dbg["qrt01"][1],
                                              in_=QRT[:, 1, :])

                # ------------- phase B/C: scores (causal) -> yKV
                # NB: matmul start=True clears accumulate flags for the WHOLE
                # psum bank, so every concurrently-accumulating group needs its
                # own bank-padded tile.
                ykvln = lctx.enter_context(tc.tile_pool(name="ykvln", bufs=1))
                yKVt = ykvln.tile([128, 2, T], BF16, name="yKVt", tag="yKVt")
                with ExitStack() as bctx:
                    scps = bctx.enter_context(
                        tc.tile_pool(name="scps", bufs=2, space="PSUM"))
                    ykvps = bctx.enter_context(
                        tc.tile_pool(name="ykvps", bufs=4, space="PSUM"))
                    scsb = bctx.enter_context(tc.tile_pool(name="scsb", bufs=3))
                    bpsum = bctx.enter_context(
                        tc.tile_pool(name="bpsum", bufs=2, space="PSUM"))
                    ykv_td = ykvln.tile([128, TBLK, D], BF16, name="ykv_td",
                                        tag="ykv_td")
                    for c in range(2):
                        t_base = c * 512
                        ykv_tiles = {
                            tb: ykvps.tile([128, D], F32, name=f"ykv_{tb}",
                                           tag="ykv_acc")
                            for tb in range(4 * c, 4 * c + 4)
                        }
                        for sb in range(4 * c + 4):
                            s0 = sb * 128
                            diag = s0 >= t_base
                            t_lo = s0 if diag else t_base
                            w = t_base + 512 - t_lo
                            ps = scps.tile([128, 512], F32, name="sc_ps",
                                           tag="sc_ps")
                            for ntl in range(NTILES):
                                nc.tensor.matmul(
                                    ps[:, :w],
                                    lhsT=QRT[:, ntl, s0:s0 + 128],
                                    rhs=QRT[:, ntl, t_lo:t_lo + w],
                                    start=(ntl == 0), stop=(ntl == NTILES - 1))
                            sc_sb = scsb.tile([128, 512], BF16, name="sc_sb",
                                              tag="sc_sb")
                            if diag:
                                nc.vector.tensor_mul(sc_sb[:, 0:128],
                                                     ps[:, 0:128], trim)
                                if w > 128:
                                    nc.scalar.activation(
                                        out=sc_sb[:, 128:w], in_=ps[:, 128:w],
                                        func=AF.Copy)
                            else:
                                nc.scalar.activation(out=sc_sb[:, 0:w],
                                                     in_=ps[:, 0:w], func=AF.Copy)
                            if dbg and li == 0 and c == 0 and sb == 0:
                                nc.sync.dma_start(out=dbg["sc"], in_=sc_sb)
                            for q in range(w // 128):
                                tb = (t_lo // 128) + q
                                nc.tensor.matmul(
                                    ykv_tiles[tb],
                                    lhsT=sc_sb[:, q * 128:(q + 1) * 128],
                                    rhs=x_bf[:, sb, :],
                                    start=(sb == 0), stop=(sb == tb),
                                    skip_group_check=True)
                        for tb in range(4 * c, 4 * c + 4):
                            layer_norm_tile(ykv_td[:, tb, :], ykv_tiles[tb])
                    for i in range(TBLK):
                        for dh in range(2):
                            transpose_128(
                                yKVt[:, dh, i * 128:(i + 1) * 128],
                                ykv_td[:, i, dh * 128:(dh + 1) * 128],
                                bpsum,
                            )
                    if dbg and li == 0:
                        nc.sync.dma_start(out=dbg["ykv"], in_=ykv_td)
                        nc.sync.dma_start(out=dbg["ykvt"], in_=yKVt)

            # ---------------- phase D/E: y_sparse, m = xsp*ysp, yMLP
            # (still inside lctx: m tiles overwrite the QRT storage, which is
            # fully consumed by phase B)
                m_t = QRT  # alias: phase D writes m into the QRT region
                with ExitStack() as dctx:
                    encvs = dctx.enter_context(tc.tile_pool(name="encvs", bufs=4))
                    xspr = dctx.enter_context(tc.tile_pool(name="xspr", bufs=4))
                    ysps = dctx.enter_context(
                        tc.tile_pool(name="ysps", bufs=2, space="PSUM"))
                    for ntl in range(NTILES):
                        encv_c = encvs.tile([128, 2, 128], BF16, name="encv_c",
                                            tag="encv_c")
                        nc.sync.dma_start(
                            out=encv_c,
                            in_=encv_r[:, :, ntl * 128:(ntl + 1) * 128])
                        ys = ysps.tile([128, T], F32, name="ys_ps", tag="ys_ps")
                        for th in range(2):
                            nc.tensor.matmul(
                                ys[:, th * 512:(th + 1) * 512],
                                lhsT=encv_c[:, 0, :],
                                rhs=yKVt[:, 0, th * 512:(th + 1) * 512],
                                start=True, stop=False)
                            nc.tensor.matmul(
                                ys[:, th * 512:(th + 1) * 512],
                                lhsT=encv_c[:, 1, :],
                                rhs=yKVt[:, 1, th * 512:(th + 1) * 512],
                                start=False, stop=True)
                        if RECOMPUTE_XSP:
                            enc_c2 = encvs.tile([128, 2, 128], BF16,
                                                name="enc_c2", tag="enc_c2")
                            nc.sync.dma_start(
                                out=enc_c2,
                                in_=enc_r[:, :, ntl * 128:(ntl + 1) * 128])
                            xs_ps = ysps.tile([128, T], F32, name="xs_ps",
                                              tag="xs_ps")
                            for th in range(2):
                                nc.tensor.matmul(
                                    xs_ps[:, th * 512:(th + 1) * 512],
                                    lhsT=enc_c2[:, 0, :],
                                    rhs=xT[:, 0, th * 512:(th + 1) * 512],
                                    start=True, stop=False)
                                nc.tensor.matmul(
                                    xs_ps[:, th * 512:(th + 1) * 512],
                                    lhsT=enc_c2[:, 1, :],
                                    rhs=xT[:, 1, th * 512:(th + 1) * 512],
                                    start=False, stop=True)
                            xspt = xspr.tile([128, T], BF16, name="xspt",
                                             tag="xspt")
                            for th in range(2):
                                nc.scalar.activation(
                                    out=xspt[:, th * 512:(th + 1) * 512],
                                    in_=xs_ps[:, th * 512:(th + 1) * 512],
                                    func=AF.Relu)
                        elif not ABLATE_DMA or ntl == 0:
                            xspt = xspr.tile([128, T], BF16, name="xspt",
                                             tag="xspt")
                            nc.sync.dma_start(out=xspt, in_=xsp_d[ntl])
                            abl_xspt = xspt
                        else:
                            xspt = abl_xspt
                        for th in range(2):
                            nc.scalar.activation(
                                out=m_t[:, ntl, th * 512:(th + 1) * 512],
                                in_=ys[:, th * 512:(th + 1) * 512], func=AF.Relu)
                        nc.vector.tensor_mul(m_t[:, ntl, :], m_t[:, ntl, :],
                                             xspt)
                        if dbg and li == 0 and ntl == 0:
                            nc.sync.dma_start(out=dbg["m0"], in_=m_t[:, ntl, :])
                with ExitStack() as ectx:
                    decs = ectx.enter_context(tc.tile_pool(name="decs", bufs=4))
                    empool = ectx.enter_context(
                        tc.tile_pool(name="empool", bufs=4, space="PSUM"))
                    ymlp_dt = [
                        empool.tile([128, 512], F32, name=f"ymlpdt_{q}",
                                    tag="ymlp_dt")
                        for q in range(4)
                    ]
                    for ntl in range(NTILES):
                        dec_c = decs.tile([128, D], BF16, name="dec_c",
                                          tag="dec_c")
                        nc.sync.dma_start(out=dec_c, in_=dec_r[:, ntl, :])
                        for dh in range(2):
                            for tcq in range(2):
                                nc.tensor.matmul(
                                    ymlp_dt[dh * 2 + tcq],
                                    lhsT=dec_c[:, dh * 128:(dh + 1) * 128],
                                    rhs=m_t[:, ntl, tcq * 512:(tcq + 1) * 512],
                                    start=(ntl == 0), stop=(ntl == NTILES - 1),
                                    skip_group_check=True)
                    stageT = stagep.tile([128, 2, T], F32, name="stageT",
                                         tag="stageT")
                    for dh in range(2):
                        for tcq in range(2):
                            nc.scalar.copy(
                                out=stageT[:, dh, tcq * 512:(tcq + 1) * 512],
                                in_=ymlp_dt[dh * 2 + tcq])

            # ---------------- phase F: all-reduce + transpose + x update
            cc_in = dram.tile([D, T], F32, name="cc_in", tag="cc_in")
            cc_out = dram.tile([D, T], F32, name="cc_out", tag="cc_out")
            cc_in_r = cc_in.rearrange("(dh p) t -> p dh t", p=128)
            cc_out_r = cc_out.rearrange("(dh p) t -> p dh t", p=128)
            nc.sync.dma_start(out=cc_in_r, in_=stageT)
            if ABLATE_CC:
                nc.sync.dma_start(out=cc_out, in_=cc_in)
            else:
                nc.gpsimd.collective_compute(
                    "AllReduce", ALU.add, replica_groups=REPLICA_GROUPS,
                    ins=[cc_in.opt()], outs=[cc_out.opt()])
            nc.sync.dma_start(out=stageT, in_=cc_out_r)

            xn = xpool.tile([128, TBLK, D], F32, name="x", tag="x")
            with tc.tile_pool(name="ftp", bufs=2, space="PSUM") as ftp:
                for i in range(TBLK):
                    ln_in = lnp.tile([128, D], F32, name="ln_in", tag="ln_in")
                    for dh in range(2):
                        pst = ftp.tile([128, 128], F32, name="ftp_t",
                                       tag="ftp_t")
                        nc.tensor.transpose(
                            out=pst,
                            in_=stageT[:, dh, i * 128:(i + 1) * 128],
                            identity=ident32)
                        nc.scalar.copy(
                            out=ln_in[:, dh * 128:(dh + 1) * 128], in_=pst)
                    layer_norm_tile(ln_in, ln_in)
                    nc.vector.tensor_add(xn[:, i, :], x_f32[:, i, :], ln_in)
                    layer_norm_tile(xn[:, i, :], xn[:, i, :])
            if dbg and li == 0:
                nc.sync.dma_start(out=dbg["x1"], in_=xn)
            with tc.tile_pool(name="fpsum", bufs=2, space="PSUM") as fpsum:
                x_bf, xT = make_x_forms(xn, fpsum)
            x_f32 = xn

        # ---------------- final logits
        with tc.tile_pool(name="lps", bufs=2, space="PSUM") as lps, \
                tc.tile_pool(name="lout", bufs=2) as lout:
            for i in range(TBLK):
                ps = lps.tile([128, VOCAB], F32, name="l_ps", tag="l_ps")
                nc.tensor.matmul(ps, lhsT=xT[:, 0, i * 128:(i + 1) * 128],
                                 rhs=lmh_t[:, 0, :], start=True, stop=False)
                nc.tensor.matmul(ps, lhsT=xT[:, 1, i * 128:(i + 1) * 128],
                                 rhs=lmh_t[:, 1, :], start=False, stop=True)
                ot = lout.tile([128, VOCAB], F32, name="l_sb", tag="l_sb")
                nc.scalar.copy(out=ot, in_=ps)
                nc.sync.dma_start(out=out_r[:, i, :], in_=ot)

    _install_json_fix(nc)
    return nc


_NC_CACHE = {}


def _get_nc(n_layers=N_LAYER):
    if n_layers not in _NC_CACHE:
        _NC_CACHE[n_layers] = build_nc(n_layers)
    return _NC_CACHE[n_layers]


def prepare_in_maps(idx, encoder, encoder_v, decoder, embed, lm_head):
    idx = np.asarray(idx)
    encoder = np.asarray(encoder, dtype=np.float32)
    encoder_v = np.asarray(encoder_v, dtype=np.float32)
    decoder = np.asarray(decoder, dtype=np.float32)
    embed = np.asarray(embed, dtype=np.float32)
    lm_head = np.asarray(lm_head, dtype=np.float32)

    perm = _pair_perm()
    costab, sintab = _tables()
    lmh_bf = lm_head.astype(ml_dtypes.bfloat16)

    x0 = _ln_np(embed[idx])  # (B, T, D) f32
    dec3 = decoder.reshape(NH, N, D)

    in_maps = []
    for core in range(N_CORES):
        b, h = core // NH, core % NH
        in_maps.append({
            "x0": np.ascontiguousarray(x0[b]),
            "enc": np.ascontiguousarray(
                encoder[h][:, perm]).astype(ml_dtypes.bfloat16),
            "encv": np.ascontiguousarray(
                encoder_v[h][:, perm]).astype(ml_dtypes.bfloat16),
            "dec": np.ascontiguousarray(
                dec3[h][perm, :]).astype(ml_dtypes.bfloat16),
            "costab": costab,
            "sintab": sintab,
            "lmh": lmh_bf,
        })
    return in_maps


def kernel(idx, encoder, encoder_v, decoder, embed, lm_head, *,
           trace=False, n_layers=N_LAYER):
    nc = _get_nc(n_layers)
    in_maps = prepare_in_maps(idx, encoder, encoder_v, decoder, embed, lm_head)
    res = run_bass_kernel_spmd(nc, in_maps, core_ids=list(range(N_CORES)),
                               trace=trace)
    out = np.stack([res.results[0]["logits"], res.results[NH]["logits"]])
    kernel.last_result = res
    return out.astype(np.float32)
